# revision 13
# baseline (speedup 1.0000x reference)
"""Trainium2 Bass kernel for nn_Block_39874476376768 (dense transformer block).

Sharding: 8 cores = 2 batches x 4 ranks. Each rank computes K/V for all 2048
tokens of its batch, and owns query tiles {r, 7-r} (2x256 tokens, balanced
causal work). Host permutes tokens per rank so owned queries sit at fixed
slots [1536,2048) -> one uniform program for all cores; causality is data
(bias/mask tensors). Zero cross-core communication; host assembles shards.

All activations feature-major ([feature, token]); weights pre-transposed and
tf32-rounded on host; matmuls in float32r (full PE rate, ~1e-4 rel err).
"""

import sys

if "/opt/trn_rl_repo" not in sys.path:
    sys.path.insert(0, "/opt/trn_rl_repo")

import ml_dtypes
import numpy as np

import concourse.bass as bass
import concourse.tile as tile
from concourse import bacc, mybir
from concourse.bass_utils import run_bass_kernel_spmd

F32 = mybir.dt.float32
F32R = mybir.dt.float32r
AFT = mybir.ActivationFunctionType
ALU = mybir.AluOpType

S, E, H, D, F = 2048, 1024, 16, 64, 4096
EC = E // 128            # 8 e-chunks
TT = S // 512            # 4 token 512-tiles
NOWN = 512               # owned tokens per core
NEG = -float(2 ** 20)    # additive mask value (exact in tf32)
LN_EPS = 1e-5
ATT_SCALE = 1.0 / np.sqrt(D)

_PROGRAM_CACHE = {}


def _round_tf32(x):
    """Round fp32 array to float32r (tf32-like, 10-bit mantissa), RNE."""
    x = np.ascontiguousarray(x, dtype=np.float32)
    u = x.view(np.uint32).copy()
    lsb = (u >> np.uint32(13)) & np.uint32(1)
    u += np.uint32(4095) + lsb
    u &= np.uint32(0xFFFFE000)
    out = u.view(np.float32).copy()
    out[~np.isfinite(x)] = x[~np.isfinite(x)]
    return out


def _build_program():
    """Build the uniform per-core program. Returns compiled Bacc."""
    nc = bacc.Bacc("TRN2", target_bir_lowering=False, debug=False, num_devices=8)

    # ---- DRAM I/O ----
    xhT_d = nc.dram_tensor("xhT", [E, S], F32, kind="ExternalInput")
    xrT_d = nc.dram_tensor("xrT", [E, S], F32, kind="ExternalInput")
    # Q,K weight blocks: [16 fchunks (0-7 Q, 8-15 K), 8 echunks, 128, 128]
    wqk_d = nc.dram_tensor("wqkT", [16, EC, 128, 128], F32R, kind="ExternalInput")
    # V weights: [8 echunks, 128, 1024 vfeats]
    wv_d = nc.dram_tensor("wvT", [EC, 128, E], F32R, kind="ExternalInput")
    # out_proj blocks: [8 ehchunks, 8 eochunks, 128, 128]
    ow_d = nc.dram_tensor("owT", [EC, EC, 128, 128], F32R, kind="ExternalInput")
    fc1_d = nc.dram_tensor("fc1T", [F // 128, EC, 128, 128], F32R, kind="ExternalInput")
    fc2_d = nc.dram_tensor("fc2T", [EC, F // 128, 128, 128], F32R, kind="ExternalInput")
    maskA_d = nc.dram_tensor("maskA", [128, 256], F32, kind="ExternalInput")
    maskC_d = nc.dram_tensor("maskC", [128, 256], F32, kind="ExternalInput")
    qbias_d = nc.dram_tensor("qbias", [1, 12 * 512], mybir.dt.bfloat16, kind="ExternalInput")
    r2T_d = nc.dram_tensor("r2T", [E, NOWN], F32R, kind="ExternalOutput")
    yT_d = nc.dram_tensor("yT", [E, NOWN], F32, kind="ExternalOutput")

    with tile.TileContext(nc) as tc:
        _emit(nc, tc, locals())
    nc.compile()
    return nc


def _emit(nc, tc, d):
    xhT_d, xrT_d = d["xhT_d"], d["xrT_d"]
    wqk_d, wv_d, ow_d = d["wqk_d"], d["wv_d"], d["ow_d"]
    fc1_d, fc2_d = d["fc1_d"], d["fc2_d"]
    maskA_d, maskC_d, qbias_d = d["maskA_d"], d["maskC_d"], d["qbias_d"]
    r2T_d, yT_d = d["r2T_d"], d["yT_d"]
    BF16 = mybir.dt.bfloat16

    # internal DRAM spill for hT
    hTd = nc.dram_tensor("hTd", [EC, 128, S], F32R)

    from contextlib import ExitStack

    ctx = ExitStack()
    with ctx:
        glob = ctx.enter_context(tc.tile_pool(name="glob", bufs=1))
        maskA = glob.tile([128, 256], F32, tag="maskA")
        maskC = glob.tile([128, 256], F32, tag="maskC")
        qbias = glob.tile([1, 12 * 512], BF16, tag="qbias")
        ones_col = glob.tile([128, 1], F32R, tag="ones_col")
        ones_row = glob.tile([1, 128], BF16, tag="ones_row")
        ones8 = glob.tile([128, 8], F32, tag="ones8")
        tmp1 = glob.tile([1, 128], F32, tag="tmp_ones")
        eps1 = glob.tile([1, 1], F32, tag="eps1")

        nc.sync.dma_start(out=maskA[:], in_=maskA_d[:])
        nc.sync.dma_start(out=maskC[:], in_=maskC_d[:])
        nc.sync.dma_start(out=qbias[:], in_=qbias_d[:])
        nc.vector.memset(tmp1[:], 1.0)
        nc.vector.memset(ones8[:], 1.0)
        nc.vector.memset(eps1[:], LN_EPS)
        nc.scalar.activation(ones_col[:], ones8[:, 0:1], AFT.Copy, bias=0.0, scale=1.0)
        nc.scalar.activation(ones_row[:], tmp1[:], AFT.Copy, bias=0.0, scale=1.0)

        rop = ctx.enter_context(tc.tile_pool(name="rop", bufs=1))
        resid_own = [rop.tile([128, 512], F32R, tag=f"ro{k}", name=f"ro{k}")
                     for k in range(EC)]
        ctxT = [rop.tile([128, NOWN], F32R, tag=f"ctx{k}", name=f"ctx{k}")
                for k in range(EC)]

        # ================= Stage A: x = xh + xr, LN1 -> hT (DRAM) ===========
        with tc.tile_pool(name="lnio", bufs=4) as lnio, \
             tc.tile_pool(name="lnsq", bufs=3) as lnsq, \
             tc.tile_pool(name="lnps", bufs=2, space="PSUM") as lnps, \
             tc.tile_pool(name="lnst", bufs=4) as lnst, \
             tc.tile_pool(name="lnbc", bufs=3) as lnbc, \
             tc.tile_pool(name="hto", bufs=4) as hto:
            for T in range(TT):
                cols = slice(T * 512, T * 512 + 512)
                xT = []
                s_ps = lnps.tile([1, 512], F32, tag="s")
                q_ps = lnps.tile([1, 512], F32, tag="q")
                for k in range(EC):
                    xh = lnio.tile([128, 512], F32, tag="xh")
                    xr = lnio.tile([128, 512], F32, tag="xr")
                    nc.sync.dma_start(out=xh[:], in_=xhT_d[k * 128:(k + 1) * 128, cols])
                    nc.sync.dma_start(out=xr[:], in_=xrT_d[k * 128:(k + 1) * 128, cols])
                    if T == TT - 1:
                        x = resid_own[k]
                    else:
                        x = lnio.tile([128, 512], F32R, tag=f"x{k}", bufs=2,
                                      name=f"x{k}_{T}")
                    nc.vector.tensor_add(x[:], xh[:], xr[:])
                    xT.append(x)
                    sq = lnsq.tile([128, 512], F32R, tag="sq")
                    nc.scalar.activation(sq[:], x[:], AFT.Square, bias=0.0, scale=1.0)
                    nc.tensor.matmul(s_ps[:], ones_col[:], x[:],
                                     start=(k == 0), stop=(k == EC - 1))
                    nc.tensor.matmul(q_ps[:], ones_col[:], sq[:],
                                     start=(k == 0), stop=(k == EC - 1))
                mu = lnst.tile([1, 512], F32, tag="mu")
                va = lnst.tile([1, 512], F32, tag="va")
                rs = lnst.tile([1, 512], F32, tag="rs")
                musq = lnst.tile([1, 512], F32, tag="musq")
                nc.vector.tensor_scalar_mul(mu[:], s_ps[:], 1.0 / E)
                nc.scalar.activation(musq[:], mu[:], AFT.Square, bias=0.0, scale=1.0)
                nc.vector.scalar_tensor_tensor(
                    out=va[:], in0=q_ps[:], scalar=1.0 / E, in1=musq[:],
                    op0=ALU.mult, op1=ALU.subtract)
                nc.scalar.activation(va[:], va[:], AFT.Sqrt, bias=eps1[:], scale=1.0)
                nc.vector.reciprocal(rs[:], va[:])
                mu_bc = lnbc.tile([128, 512], F32, tag="mu_bc")
                rs_bc = lnbc.tile([128, 512], F32, tag="rs_bc")
                nc.gpsimd.partition_broadcast(mu_bc[:], mu[:])
                nc.gpsimd.partition_broadcast(rs_bc[:], rs[:])
                for k in range(EC):
                    t = lnsq.tile([128, 512], F32, tag="cent")
                    nc.vector.tensor_sub(t[:], xT[k][:], mu_bc[:])
                    ho = hto.tile([128, 512], F32R, tag="ho")
                    nc.vector.tensor_mul(ho[:], t[:], rs_bc[:])
                    nc.sync.dma_start(out=hTd[k, :, cols], in_=ho[:])

        # V' ones columns (col 64 of each 65-stride head slot)
        kv_stack = ExitStack()
        kvp = kv_stack.enter_context(tc.tile_pool(name="kvp", bufs=1))
        kT = [kvp.tile([128, S], F32R, tag=f"kT{k}", name=f"kT{k}") for k in range(4)]
        vP = kvp.tile([128, 16 * 520], F32R, tag="vP")
        qT = [kvp.tile([128, NOWN], F32R, tag=f"qT{k}", name=f"qT{k}")
              for k in range(EC)]
        for c in range(16):
            dst = vP[:, c * 520:(c + 1) * 520] \
                .rearrange("p (h x) -> p h x", h=8)[:, :, 64:65]
            nc.scalar.activation(dst, ones8[:], AFT.Copy, bias=0.0, scale=1.0)

        # ============ Stages B+C per half: QKV + attention ============
        for half in range(2):
            with tc.tile_pool(name="hw", bufs=1) as hwp, \
                 tc.tile_pool(name="wblk", bufs=6) as wblk, \
                 tc.tile_pool(name="mmps", bufs=3, space="PSUM") as mmps:
                for T in range(TT):
                    cols = slice(T * 512, T * 512 + 512)
                    hw = [hwp.tile([128, 512], F32R, tag=f"hw{k}", bufs=2,
                                   name=f"hw{half}_{T}_{k}") for k in range(EC)]
                    for k in range(EC):
                        nc.sync.dma_start(out=hw[k][:], in_=hTd[k, :, cols])
                    # K section
                    for fc in range(4):
                        fblk = 8 + half * 4 + fc
                        ps = mmps.tile([128, 512], F32, tag="ps")
                        for k in range(EC):
                            w = wblk.tile([128, 128], F32R, tag="w")
                            nc.sync.dma_start(out=w[:], in_=wqk_d[fblk, k])
                            nc.tensor.matmul(ps[:], w[:], hw[k][:],
                                             start=(k == 0), stop=(k == EC - 1))
                        nc.scalar.activation(kT[fc][:, cols], ps[:],
                                             AFT.Copy, bias=0.0, scale=1.0)
                    # V section (4 token chunks in this window)
                    for tl in range(4):
                        tch = T * 4 + tl
                        ps = mmps.tile([128, 512], F32, tag="ps")
                        for k in range(EC):
                            wv = wblk.tile([128, 512], F32R, tag="wv")
                            nc.sync.dma_start(
                                out=wv[:],
                                in_=wv_d[k, :, half * 512:(half + 1) * 512])
                            nc.tensor.matmul(
                                ps[:], hw[k][:, tl * 128:(tl + 1) * 128], wv[:],
                                start=(k == 0), stop=(k == EC - 1))
                        dst = vP[:, tch * 520:(tch + 1) * 520] \
                            .rearrange("p (h x) -> p h x", h=8)[:, :, 0:64]
                        nc.scalar.activation(dst, ps[:], AFT.Copy,
                                             bias=0.0, scale=1.0)
                    # Q section (own tokens live in last window)
                    if T == TT - 1:
                        for fc in range(4):
                            fblk = half * 4 + fc
                            ps = mmps.tile([128, 512], F32, tag="ps")
                            for k in range(EC):
                                w = wblk.tile([128, 128], F32R, tag="w")
                                nc.sync.dma_start(out=w[:], in_=wqk_d[fblk, k])
                                nc.tensor.matmul(ps[:], w[:], hw[k][:],
                                                 start=(k == 0),
                                                 stop=(k == EC - 1))
                            nc.scalar.activation(qT[half * 4 + fc][:], ps[:],
                                                 AFT.Copy, bias=0.0, scale=1.0)

            # ---- attention for this half's 8 heads ----
            with tc.tile_pool(name="scps", bufs=2, space="PSUM") as scps, \
                 tc.tile_pool(name="ctxps", bufs=2, space="PSUM") as ctxps, \
                 tc.tile_pool(name="expp", bufs=3) as expp, \
                 tc.tile_pool(name="nrm", bufs=4) as nrm:
                for l in range(8):
                    kTh = kT[l // 2][64 * (l % 2):64 * (l % 2) + 64, :]
                    qTh = qT[half * 4 + l // 2][64 * (l % 2):64 * (l % 2) + 64, :]
                    ctx_ps = ctxps.tile([65, 512], F32, tag="ctx")
                    for c in range(16):
                        vPh = vP[:, c * 520 + l * 65: c * 520 + l * 65 + 65]
                        if c < 14:
                            ps = scps.tile([128, 512], F32, tag="sc")
                            nc.tensor.matmul(ps[:], kTh[:, c * 128:(c + 1) * 128],
                                             qTh[:], start=True, stop=(c >= 12))
                            if c < 6:
                                nc.tensor.matmul(ps[:, 0:256], ones_row[:],
                                                 qbias[:, c * 512:c * 512 + 256],
                                                 start=False, stop=True,
                                                 skip_group_check=True)
                            elif c < 12:
                                nc.tensor.matmul(ps[:], ones_row[:],
                                                 qbias[:, c * 512:(c + 1) * 512],
                                                 start=False, stop=True,
                                                 skip_group_check=True)
                            if c == 12:
                                nc.vector.tensor_add(ps[:, 0:256], ps[:, 0:256],
                                                     maskA[:])
                            elif c == 13:
                                nc.vector.tensor_add(ps[:, 0:256], ps[:, 0:256],
                                                     maskC[:])
                            ex = expp.tile([128, 512], F32R, tag="ex")
                            nc.scalar.activation(ex[:], ps[:], AFT.Exp,
                                                 bias=0.0, scale=ATT_SCALE)
                            nc.tensor.matmul(ctx_ps[:], vPh, ex[:],
                                             start=(c == 0), stop=False,
                                             skip_group_check=True)
                        else:
                            ps = scps.tile([128, 256], F32, tag="sc2")
                            nc.tensor.matmul(ps[:], kTh[:, c * 128:(c + 1) * 128],
                                             qTh[:, 256:512], start=True, stop=True)
                            nc.vector.tensor_add(
                                ps[:], ps[:], maskA[:] if c == 14 else maskC[:])
                            ex = expp.tile([128, 256], F32R, tag="ex2")
                            nc.scalar.activation(ex[:], ps[:], AFT.Exp,
                                                 bias=0.0, scale=ATT_SCALE)
                            nc.tensor.matmul(ctx_ps[:, 256:512], vPh, ex[:],
                                             start=False, stop=(c == 15),
                                             skip_group_check=True)
                    rec = nrm.tile([1, 512], F32, tag="rec")
                    nc.vector.reciprocal(rec[:], ctx_ps[64:65, :])
                    rec_bc = nrm.tile([64, 512], F32, tag="rec_bc")
                    nc.gpsimd.partition_broadcast(rec_bc[:], rec[:])
                    h = half * 8 + l
                    dst = ctxT[h // 2][64 * (h % 2):64 * (h % 2) + 64, :]
                    nc.vector.tensor_mul(dst, ctx_ps[0:64, :], rec_bc[:])
        kv_stack.close()

        # ============ Stage D: out-proj + residual; E: LN2; F: MLP ==========
        r2p = ctx.enter_context(tc.tile_pool(name="r2p", bufs=1))
        resid2T = [r2p.tile([128, 512], F32R, tag=f"r2{k}", name=f"r2{k}")
                   for k in range(EC)]
        h2T = [r2p.tile([128, 512], F32R, tag=f"h2{k}", name=f"h2{k}")
               for k in range(EC)]

        with tc.tile_pool(name="owblk", bufs=6) as owblk, \
             tc.tile_pool(name="prps", bufs=3, space="PSUM") as prps:
            for eo in range(EC):
                ps = prps.tile([128, 512], F32, tag="pr")
                for k in range(EC):
                    w = owblk.tile([128, 128], F32R, tag="ow")
                    nc.sync.dma_start(out=w[:], in_=ow_d[k, eo])
                    nc.tensor.matmul(ps[:], w[:], ctxT[k][:],
                                     start=(k == 0), stop=(k == EC - 1))
                nc.vector.tensor_add(resid2T[eo][:], ps[:], resid_own[eo][:])
                nc.sync.dma_start(out=r2T_d[eo * 128:(eo + 1) * 128, :],
                                  in_=resid2T[eo][:])

        with tc.tile_pool(name="l2sq", bufs=3) as l2sq, \
             tc.tile_pool(name="l2ps", bufs=2, space="PSUM") as l2ps, \
             tc.tile_pool(name="l2st", bufs=2) as l2st, \
             tc.tile_pool(name="l2bc", bufs=2) as l2bc:
            s_ps = l2ps.tile([1, 512], F32, tag="s")
            q_ps = l2ps.tile([1, 512], F32, tag="q")
            for k in range(EC):
                sq = l2sq.tile([128, 512], F32R, tag="sq")
                nc.scalar.activation(sq[:], resid2T[k][:], AFT.Square,
                                     bias=0.0, scale=1.0)
                nc.tensor.matmul(s_ps[:], ones_col[:], resid2T[k][:],
                                 start=(k == 0), stop=(k == EC - 1))
                nc.tensor.matmul(q_ps[:], ones_col[:], sq[:],
                                 start=(k == 0), stop=(k == EC - 1))
            mu = l2st.tile([1, 512], F32, tag="mu")
            va = l2st.tile([1, 512], F32, tag="va")
            rs = l2st.tile([1, 512], F32, tag="rs")
            musq = l2st.tile([1, 512], F32, tag="musq")
            nc.vector.tensor_scalar_mul(mu[:], s_ps[:], 1.0 / E)
            nc.scalar.activation(musq[:], mu[:], AFT.Square, bias=0.0, scale=1.0)
            nc.vector.scalar_tensor_tensor(out=va[:], in0=q_ps[:], scalar=1.0 / E,
                                           in1=musq[:], op0=ALU.mult,
                                           op1=ALU.subtract)
            nc.scalar.activation(va[:], va[:], AFT.Sqrt, bias=eps1[:], scale=1.0)
            nc.vector.reciprocal(rs[:], va[:])
            mu_bc = l2bc.tile([128, 512], F32, tag="mu_bc")
            rs_bc = l2bc.tile([128, 512], F32, tag="rs_bc")
            nc.gpsimd.partition_broadcast(mu_bc[:], mu[:])
            nc.gpsimd.partition_broadcast(rs_bc[:], rs[:])
            for k in range(EC):
                t = l2sq.tile([128, 512], F32, tag="cent")
                nc.vector.tensor_sub(t[:], resid2T[k][:], mu_bc[:])
                nc.vector.tensor_mul(h2T[k][:], t[:], rs_bc[:])

        with tc.tile_pool(name="f1blk", bufs=8) as f1blk, \
             tc.tile_pool(name="aTp", bufs=1) as aTp, \
             tc.tile_pool(name="f1ps", bufs=3, space="PSUM") as f1ps:
            aT = [aTp.tile([128, 512], F32R, tag=f"aT{f}", name=f"aT{f}")
                  for f in range(F // 128)]
            for fc in range(F // 128):
                ps = f1ps.tile([128, 512], F32, tag="f1")
                for k in range(EC):
                    w = f1blk.tile([128, 128], F32R, tag="w1")
                    nc.sync.dma_start(out=w[:], in_=fc1_d[fc, k])
                    nc.tensor.matmul(ps[:], w[:], h2T[k][:],
                                     start=(k == 0), stop=(k == EC - 1))
                nc.scalar.activation(aT[fc][:], ps[:], AFT.Gelu, bias=0.0, scale=1.0)
            with tc.tile_pool(name="f2blk", bufs=8) as f2blk, \
                 tc.tile_pool(name="f2ps", bufs=2, space="PSUM") as f2ps, \
                 tc.tile_pool(name="yout", bufs=2) as yout:
                for eo in range(EC):
                    ps = f2ps.tile([128, 512], F32, tag="f2")
                    for fc in range(F // 128):
                        w = f2blk.tile([128, 128], F32R, tag="w2")
                        nc.sync.dma_start(out=w[:], in_=fc2_d[eo, fc])
                        nc.tensor.matmul(ps[:], w[:], aT[fc][:],
                                         start=(fc == 0), stop=(fc == F // 128 - 1))
                    y = yout.tile([128, 512], F32, tag="y")
                    nc.scalar.activation(y[:], ps[:], AFT.Copy, bias=0.0, scale=1.0)
                    nc.sync.dma_start(out=yT_d[eo * 128:(eo + 1) * 128, :], in_=y[:])


def _host_prep(inputs):
    """Build per-core in_maps + assembly metadata."""
    hidden = np.asarray(inputs["hidden_states"], dtype=np.float32)
    residual = np.asarray(inputs["residual"], dtype=np.float32)
    ln1_w = np.asarray(inputs["ln1_w"], dtype=np.float32)
    ln1_b = np.asarray(inputs["ln1_b"], dtype=np.float32)
    wqkv = np.asarray(inputs["Wqkv_w"], dtype=np.float32)
    wqkv_b = np.asarray(inputs["Wqkv_b"], dtype=np.float32)
    out_w = np.asarray(inputs["out_w"], dtype=np.float32)
    out_b = np.asarray(inputs["out_b"], dtype=np.float32)
    ln2_w = np.asarray(inputs["ln2_w"], dtype=np.float32)
    ln2_b = np.asarray(inputs["ln2_b"], dtype=np.float32)
    fc1_w = np.asarray(inputs["fc1_w"], dtype=np.float32)
    fc1_b = np.asarray(inputs["fc1_b"], dtype=np.float32)
    fc2_w = np.asarray(inputs["fc2_w"], dtype=np.float32)
    fc2_b = np.asarray(inputs["fc2_b"], dtype=np.float32)

    for nm, b in (("Wqkv_b", wqkv_b), ("out_b", out_b), ("fc1_b", fc1_b),
                  ("fc2_b", fc2_b), ("ln1_b", ln1_b), ("ln2_b", ln2_b)):
        if np.any(b != 0):
            raise NotImplementedError(f"nonzero bias {nm} not supported")

    # fold LN gains into following weights
    wqkv_eff = wqkv * ln1_w[None, :]
    fc1_eff = fc1_w * ln2_w[None, :]

    # weight layouts (shared across cores)
    wqkvT = np.ascontiguousarray(wqkv_eff.T)          # [E, 3E]
    # blocks [16, 8, 128, 128]: f-blocks 0-7 = Q, 8-15 = K
    qk = wqkvT[:, :2 * E]                              # [E, 2048]
    wqk = _round_tf32(
        qk.reshape(EC, 128, 16, 128).transpose(2, 0, 1, 3))
    wv = _round_tf32(
        np.ascontiguousarray(wqkvT[:, 2 * E:]).reshape(EC, 128, E))
    owT = np.ascontiguousarray(out_w.T)                # [E(h), E(out)]
    ow = _round_tf32(owT.reshape(EC, 128, EC, 128).transpose(0, 2, 1, 3))
    fc1T = np.ascontiguousarray(fc1_eff.T)             # [E, F]
    fc1b = _round_tf32(
        fc1T.reshape(EC, 128, F // 128, 128).transpose(2, 0, 1, 3))
    fc2T = np.ascontiguousarray(fc2_w.T)               # [F, E]
    fc2b = _round_tf32(
        fc2T.reshape(F // 128, 128, EC, 128).transpose(2, 0, 1, 3))

    # masks (core-independent)
    ii = np.arange(128)[:, None]
    qq = np.arange(256)[None, :]
    maskA = np.where(qq >= ii, 0.0, NEG).astype(np.float32)
    maskC = np.where(qq >= ii + 128, 0.0, NEG).astype(np.float32)

    in_maps = []
    perms = []
    for c in range(8):
        b, r = divmod(c, 4)
        t1, t2 = r, 7 - r
        others = [j for j in range(8) if j not in (t1, t2)]
        order = others + [t1, t2]
        perm = np.concatenate([np.arange(j * 256, j * 256 + 256) for j in order])
        perms.append((b, t1, t2))
        xhT = np.ascontiguousarray(hidden[b].T[:, perm])
        xrT = np.ascontiguousarray(residual[b].T[:, perm])
        # qbias [12, 512]
        qb = np.full((12, 512), NEG, dtype=np.float32)
        for cc in range(12):
            if cc < 2 * r:
                qb[cc, 0:256] = 0.0
            if cc < 2 * (6 - r):
                qb[cc, 256:512] = 0.0
        in_maps.append({
            "xhT": xhT, "xrT": xrT, "wqkT": wqk, "wvT": wv, "owT": ow,
            "fc1T": fc1b, "fc2T": fc2b, "maskA": maskA, "maskC": maskC,
            "qbias": qb.reshape(1, -1).astype(ml_dtypes.bfloat16),
        })
    return in_maps, perms


def kernel(**inputs):
    if "prog" not in _PROGRAM_CACHE:
        _PROGRAM_CACHE["prog"] = _build_program()
    nc = _PROGRAM_CACHE["prog"]

    in_maps, perms = _host_prep(inputs)
    res = run_bass_kernel_spmd(nc, in_maps, list(range(8))).results

    B = inputs["hidden_states"].shape[0]
    y = np.empty((B, S, E), dtype=np.float32)
    r2 = np.empty((B, S, E), dtype=np.float32)
    for c in range(8):
        b, t1, t2 = perms[c]
        yT = res[c]["yT"]
        r2T = res[c]["r2T"]
        y[b, t1 * 256:(t1 + 1) * 256] = yT[:, 0:256].T
        y[b, t2 * 256:(t2 + 1) * 256] = yT[:, 256:512].T
        r2[b, t1 * 256:(t1 + 1) * 256] = r2T[:, 0:256].T
        r2[b, t2 * 256:(t2 + 1) * 256] = r2T[:, 256:512].T
    return y, r2
